# revision 33
# baseline (speedup 1.0000x reference)
"""Trainium2 Bass kernel for PoolingPMATopK.

Reference computation (per batch b, query q):
  scores[q, n] = seed[q] . x[b, n]          (n = 0..8191, h = 768)
  top-128 of scores -> softmax(vals * 12^-0.5) -> weighted sum of x rows.

Strategy per core (2 batches, batch-data-parallel over 8 cores):
  - SWDGE casting DMA loads x fp32->fp16 straight into the fp16
    natural-layout resident tile (the cast costs no engine time), the
    single pass over HBM that sets the ~160us roofline.
  - PE transposes the fp16 chunks (~80ns each vs ~340ns for fp32) into
    PSUM; ACT copies them out as the fp16 moving operand for mm1.
  - mm1 fp16 with a 32-wide stationary qT block writing a [32, 512]
    PSUM tile at partition 0; a small ACT copy + SBUF->SBUF DMA places
    each window's strip on scores partitions 32*(w%4) (engines cannot
    cross partitions, DMA cannot read PSUM).  Scores stay fp32: the
    top-128 boundary is extremely sensitive (flat softmax - every
    wrongly selected row costs ~7%), so theta must be the exact 128th
    largest of the fp32 scores as computed.
  - Permuted quarter layout: window w -> strip w%4, cols 512*(w//4);
    each 512-col group completes after 4 consecutive windows, so its
    per-row top-24 extraction (true top-128 has <= 19 per window)
    runs as full [128, 512] DVE ops hidden under the stream.
  - Exact L2 top-128 of 384 candidates -> theta, m; threshold trick:
    w = 1[s >= theta] * exp((s - m) * c); out = (w @ x) / sum(w) with a
    ones-column per chunk giving Z in the same matmul (denominator
    consistent with whatever got selected).  No gather.
  - mm2 fp16 (w and v quantization contribute ~1e-4); a 24-chunk
    overlay lets odd batches start streaming before the previous
    batch's mm2 has released the resident tile.
"""

import numpy as np

B, N, H, Q = 16, 8192, 768, 32
NCORES = 8
BPC = B // NCORES          # batches per core
NCH = N // 128             # 64 chunks of 128 rows per batch
KB = H // 128              # 6 h-blocks
WPB = N // 512             # 16 windows per batch
CW = H + 1                 # 769 resident cols per chunk (ones + data)
CSCALE = float(12 ** -0.5)
WTOP = 24                  # candidates kept per 512-col group (true max 19)
OVER = 24                  # chunks of overlay residency for odd batches
NEG = -1e30

_built = None


def _apply_patches():
    """Inline of tile_patch.py: the TileContext final Drain carries one wait
    per pending semaphore lane (walrus allows at most 1 sync wait per
    instruction on TRN2)."""
    import bass_rust as _br
    from concourse import tile as _tile
    from concourse.tile_scheduler import N_PROCS

    def _patched_drain_and_barrier(self, tick_clock, wait_clock):
        sems = self.sems.allocated()
        gc = tick_clock.global_clock
        for p in range(N_PROCS):
            tick = gc[p]
            if tick <= 0:
                continue
            sem = sems.get(p)
            if sem is None:
                continue
            self.nc.sync.wait_ge(sem, _br.tick_to_sem(tick, p))
        self.nc.sync.drain()
        self.nc.all_engine_barrier()
        assert self.sems is not None
        popped = self.nc._tile_sem_poison_stack.pop()
        assert popped is self._sem_poison
        self.nc.clear_and_free_semaphores(list(self.sems.allocated().values()))
        self.nc.all_engine_barrier()

    _tile.TileContext._drain_and_barrier = _patched_drain_and_barrier


def _build():
    import concourse.bass as bass
    import concourse.tile as tile
    from concourse import mybir

    _apply_patches()

    F32 = mybir.dt.float32
    F16 = mybir.dt.float16
    COPY = mybir.ActivationFunctionType.Copy
    EXP = mybir.ActivationFunctionType.Exp

    nc = bass.Bass()
    x_d = nc.declare_dram_parameter("x", [BPC * N, H], F32, isOutput=False)
    qT_d = nc.declare_dram_parameter("seedT", [H, Q], F32, isOutput=False)
    id_d = nc.declare_dram_parameter("ident", [128, 128], F32, isOutput=False)
    out_d = nc.declare_dram_parameter("out", [BPC * Q, H], F32, isOutput=True)

    with tile.TileContext(nc) as tc:
        with (
            tc.tile_pool(name="const", bufs=1) as cpool,
            tc.tile_pool(name="xt", bufs=2) as xtpool,
            tc.tile_pool(name="sc", bufs=2) as scpool,
            tc.tile_pool(name="work", bufs=1) as wpool,
            tc.tile_pool(name="ps_tp", bufs=2, space="PSUM") as ps_tp,
            tc.tile_pool(name="ps_m", bufs=2, space="PSUM") as ps_m,
            tc.tile_pool(name="ps_2", bufs=1, space="PSUM") as ps_2,
        ):
            id_t = cpool.tile([128, 128], F32)
            nc.sync.dma_start(id_t[:], id_d[:])
            id16_t = cpool.tile([128, 128], F16)
            nc.vector.tensor_copy(id16_t[:], id_t[:])

            qT_f32 = cpool.tile([128, KB * 32], F32)
            for k in range(KB):
                nc.sync.dma_start(
                    qT_f32[:, 32 * k:32 * k + 32], qT_d[128 * k:128 * k + 128, :]
                )
            qT_t = cpool.tile([128, KB * 32], F16)
            nc.vector.tensor_copy(qT_t[:], qT_f32[:])

            res_t = wpool.tile([128, NCH * CW], F16)
            nc.vector.memset(res_t[:, 0:NCH * CW:CW], 1.0)
            res2_t = wpool.tile([128, OVER * CW], F16)
            nc.vector.memset(res2_t[:, 0:OVER * CW:CW], 1.0)
            strip_sb = [
                wpool.tile([32, 512], F32, name=f"strip{i}") for i in range(2)
            ]

            scratch_t = wpool.tile([128, 2048], F32)
            ge_t = wpool.tile([128, 2048], F16)
            cand_t = wpool.tile([128, 4 * WTOP], F32)
            cand2_t = wpool.tile([32, 16 * WTOP], F32)
            top_t = wpool.tile([32, 128], F32)
            mth_t = wpool.tile([32, 2], F32)
            b_mth = wpool.tile([128, 2], F32)
            negcm = wpool.tile([128, 1], F32)
            rz_t = wpool.tile([32, 1], F32)
            o2_t = wpool.tile([32, H], F32)
            wT_sb = [
                wpool.tile([128, 512], F16, name=f"wT_sb{u}") for u in range(4)
            ]

            def res_chunk(bb, c):
                """Residency slice [128, CW] for chunk c of batch bb."""
                if bb % 2 == 1 and c < OVER:
                    return res2_t[:, CW * c:CW * c + CW]
                return res_t[:, CW * c:CW * c + CW]

            def res_win(bb, w):
                """Residency slice [128, 4*CW] for window w of batch bb
                (windows never straddle the res/res2 boundary)."""
                c0 = 4 * w
                if bb % 2 == 1 and c0 < OVER:
                    return res2_t[:, CW * c0:CW * (c0 + 4)]
                return res_t[:, CW * c0:CW * (c0 + 4)]

            for b in range(BPC):
                row0 = b * N
                sc_t = scpool.tile([128, 2048], F32, name="scores")
                # ---- Phase A: stream windows (casting DMA), PE
                #      transpose fp16 chunks, fp16 mm1 into the permuted
                #      quarter layout (window w -> strip w%4, col group
                #      w//4), per-group top-24 on DVE after each group.
                for w in range(WPB):
                    t = w // 4          # column group
                    jq = w % 4          # partition strip
                    # casting DMA (SWDGE): fp32 HBM -> fp16 residency
                    nc.gpsimd.dma_start(
                        res_win(b, w).rearrange(
                            "p (c e) -> p c e", c=4
                        )[:, :, 1:1 + H],
                        x_d[row0 + 512 * w:row0 + 512 * w + 512, :].rearrange(
                            "(c p) h -> p c h", p=128
                        ),
                    )
                    pw = ps_m.tile([32, 512], F32, name="pw")
                    xt = xtpool.tile([128, KB * 512], F16)
                    for cw in range(4):
                        c = 4 * w + cw
                        src = res_chunk(b, c)[:, 1:1 + H]
                        # transpose 6 h-blocks of the fp16 chunk
                        tp = ps_tp.tile([128, KB, 128], F16, name="tp")
                        for k in range(KB):
                            nc.tensor.matmul(
                                tp[:, k, :],
                                src[:, 128 * k:128 * k + 128],
                                id16_t[:],
                                is_transpose=True, start=True, stop=True,
                                skip_group_check=True,
                            )
                        dst = xt[:, 0:KB * 512].rearrange(
                            "p (k i) -> p k i", k=KB
                        )[:, :, 128 * cw:128 * cw + 128]
                        nc.scalar.activation(dst, tp[:], COPY)
                        # mm1: 6 fp16 matmuls, M=32 at partition 0
                        for k in range(KB):
                            nc.tensor.matmul(
                                pw[:, 128 * cw:128 * cw + 128],
                                qT_t[:, 32 * k:32 * k + 32],
                                xt[:, 512 * k + 128 * cw:
                                   512 * k + 128 * cw + 128],
                                start=(k == 0), stop=(k == KB - 1),
                                skip_group_check=True,
                            )
                    # scores strip: PSUM -> SBUF staging (ACT), then a
                    # small SBUF->SBUF DMA places it on partition strip
                    # 32*jq of the scores tile (engines can't cross
                    # partitions; DMA can't read PSUM).
                    cs = slice(512 * t, 512 * t + 512)
                    if jq == 0:
                        nc.scalar.activation(sc_t[0:32, cs], pw[:], COPY)
                    else:
                        sb = strip_sb[w % 2]
                        nc.scalar.activation(sb[:], pw[:], COPY)
                        nc.sync.dma_start(
                            sc_t[32 * jq:32 * jq + 32, cs], sb[:]
                        )
                    if jq == 3:
                        # column group complete: top-24 per row on [128,512]
                        cnd = cand_t[:, WTOP * t:WTOP * t + WTOP]
                        nc.vector.max(cnd[:, 0:8], sc_t[:, cs])
                        nc.vector.match_replace(
                            scratch_t[:, cs], cnd[:, 0:8], sc_t[:, cs], NEG
                        )
                        nc.vector.max(cnd[:, 8:16], scratch_t[:, cs])
                        nc.vector.match_replace(
                            scratch_t[:, cs], cnd[:, 8:16],
                            scratch_t[:, cs], NEG
                        )
                        nc.vector.max(cnd[:, 16:24], scratch_t[:, cs])

                # ---- Phase A2: L2 exact top-128 of 384 candidates ->
                #      theta, m; w = 1[s>=theta]*exp(c(s-m))
                L1W = 4 * WTOP
                for jj in range(4):
                    nc.sync.dma_start(
                        cand2_t[:, L1W * jj:L1W * jj + L1W],
                        cand_t[32 * jj:32 * jj + 32, :],
                    )
                for r in range(16):
                    nc.vector.max(top_t[:, 8 * r:8 * r + 8], cand2_t[:])
                    if r < 15:
                        nc.vector.match_replace(
                            cand2_t[:], top_t[:, 8 * r:8 * r + 8],
                            cand2_t[:], NEG,
                        )
                nc.vector.tensor_copy(mth_t[:, 0:1], top_t[:, 0:1])
                nc.vector.tensor_copy(mth_t[:, 1:2], top_t[:, 127:128])
                for jj in range(4):
                    nc.sync.dma_start(b_mth[32 * jj:32 * jj + 32, :], mth_t[:])
                nc.vector.tensor_scalar_mul(negcm[:], b_mth[:, 0:1], -CSCALE)

                # ---- Phases B+C pipelined per 512-col group: threshold
                #      (ACT exp + DVE mask) -> wT transposes -> mm2 MMs
                #      for that group's 16 chunks, so mm2 starts right
                #      after L2 and frees residency chunks early.
                p2a = ps_2.tile([32, 385], F32)
                p2b = ps_2.tile([32, 384], F32)
                for u in range(4):
                    cs = slice(512 * u, 512 * u + 512)
                    nc.scalar.activation(
                        scratch_t[:, cs], sc_t[:, cs], EXP,
                        bias=negcm[:], scale=CSCALE,
                    )
                    nc.vector.tensor_scalar(
                        ge_t[:, cs], sc_t[:, cs], b_mth[:, 1:2], None,
                        mybir.AluOpType.is_ge,
                    )
                    nc.vector.tensor_mul(
                        sc_t[:, cs], scratch_t[:, cs], ge_t[:, cs]
                    )
                    wtp = ps_2.tile([128, 512], F32, name="wtp")
                    for tt in range(4):
                        t = 4 * u + tt
                        nc.tensor.matmul(
                            wtp[:, 128 * tt:128 * tt + 128],
                            sc_t[:, 128 * t:128 * t + 128],
                            id_t[:],
                            is_transpose=True, start=True, stop=True,
                            skip_group_check=True,
                        )
                    nc.scalar.activation(wT_sb[u][:], wtp[:], COPY)
                    # mm2 for this group's chunks (i = 16u .. 16u+15):
                    # chunk i is window i//4 (strip (i//4)%4), block i%4.
                    for i in range(16 * u, 16 * u + 16):
                        tt = i % 4
                        jq = (i // 4) % 4
                        lhs = wT_sb[u][:,
                                       128 * tt + 32 * jq:
                                       128 * tt + 32 * jq + 32]
                        src = res_chunk(b, i)
                        nc.tensor.matmul(
                            p2a[:], lhs, src[:, 0:385],
                            start=(i == 0), stop=(i == NCH - 1),
                            skip_group_check=True,
                        )
                        nc.tensor.matmul(
                            p2b[:], lhs, src[:, 385:CW],
                            start=(i == 0), stop=(i == NCH - 1),
                            skip_group_check=True,
                        )
                nc.vector.reciprocal(rz_t[:], p2a[:, 0:1])
                nc.scalar.activation(
                    o2_t[:, 0:384], p2a[:, 1:385], COPY, scale=rz_t[:]
                )
                nc.scalar.activation(
                    o2_t[:, 384:768], p2b[:], COPY, scale=rz_t[:]
                )
                nc.sync.dma_start(out_d[Q * b:Q * b + Q, :], o2_t[:])

    # Split multi-wait instructions to the TRN2 1-wait-per-instruction limit
    # (the standard Bacc.compile() passes, skipped on the bass2jax run path).
    import bass_rust as _bass_rust
    _bass_rust.move_matmul_waits_to_ldweights(nc.m)
    _bass_rust.generate_event_semaphores(nc)
    return nc


def _get_nc():
    global _built
    if _built is None:
        _built = _build()
    return _built


def run(inputs, trace=False):
    from concourse.bass_utils import run_bass_kernel_spmd

    x = np.ascontiguousarray(np.asarray(inputs["input"], dtype=np.float32))
    seed = np.ascontiguousarray(np.asarray(inputs["seed"], dtype=np.float32))
    nc = _get_nc()
    seedT = np.ascontiguousarray(seed[0].T)
    ident = np.eye(128, dtype=np.float32)
    in_maps = []
    for c in range(NCORES):
        xb = np.ascontiguousarray(
            x[BPC * c:BPC * (c + 1)].reshape(BPC * N, H)
        )
        in_maps.append({"x": xb, "seedT": seedT, "ident": ident})
    res = run_bass_kernel_spmd(nc, in_maps, list(range(NCORES)), trace=trace)
    out = np.empty((B, Q, H), np.float32)
    for c in range(NCORES):
        out[BPC * c:BPC * (c + 1)] = res.results[c]["out"].reshape(BPC, Q, H)
    return out, res


def kernel(**inputs):
    out, _ = run(inputs)
    return out


# revision 35
# speedup vs baseline: 1.1535x; 1.1535x over previous
"""Trainium2 Bass kernel for PoolingPMATopK.

Reference computation (per batch b, query q):
  scores[q, n] = seed[q] . x[b, n]          (n = 0..8191, h = 768)
  top-128 of scores -> softmax(vals * 12^-0.5) -> weighted sum of x rows.

Strategy per core (2 batches, batch-data-parallel over 8 cores):
  - SWDGE casting DMA loads x fp32->fp16 straight into the fp16
    natural-layout resident tile (the cast costs no engine time), the
    single pass over HBM that sets the ~160us roofline.
  - PE transposes the fp16 chunks (~80ns each vs ~340ns for fp32) into
    PSUM; ACT copies them out as the fp16 moving operand for mm1.
  - mm1 fp16 with a 32-wide stationary qT block writing a [32, 512]
    PSUM tile at partition 0; a small ACT copy + SBUF->SBUF DMA places
    each window's strip on scores partitions 32*(w%4) (engines cannot
    cross partitions, DMA cannot read PSUM).  Scores stay fp32: the
    top-128 boundary is extremely sensitive (flat softmax - every
    wrongly selected row costs ~7%), so theta must be the exact 128th
    largest of the fp32 scores as computed.
  - Permuted quarter layout: window w -> strip w%4, cols 512*(w//4);
    each 512-col group completes after 4 consecutive windows, so its
    per-row top-24 extraction (true top-128 has <= 19 per window)
    runs as full [128, 512] DVE ops hidden under the stream.
  - Exact L2 top-128 of 384 candidates -> theta, m; threshold trick:
    w = 1[s >= theta] * exp((s - m) * c); out = (w @ x) / sum(w) with a
    ones-column per chunk giving Z in the same matmul (denominator
    consistent with whatever got selected).  No gather.
  - mm2 fp16 (w and v quantization contribute ~1e-4); a 24-chunk
    overlay lets odd batches start streaming before the previous
    batch's mm2 has released the resident tile.
"""

import numpy as np

B, N, H, Q = 16, 8192, 768, 32
NCORES = 8
BPC = B // NCORES          # batches per core
NCH = N // 128             # 64 chunks of 128 rows per batch
KB = H // 128              # 6 h-blocks
WPB = N // 512             # 16 windows per batch
CW = H + 1                 # 769 resident cols per chunk (ones + data)
CSCALE = float(12 ** -0.5)
WTOP = 24                  # candidates kept per 512-col group (true max 19)
OVER = 28                  # chunks of overlay residency for odd batches
NEG = -1e30

_built = None


def _apply_patches():
    """Inline of tile_patch.py: the TileContext final Drain carries one wait
    per pending semaphore lane (walrus allows at most 1 sync wait per
    instruction on TRN2)."""
    import bass_rust as _br
    from concourse import tile as _tile
    from concourse.tile_scheduler import N_PROCS

    def _patched_drain_and_barrier(self, tick_clock, wait_clock):
        sems = self.sems.allocated()
        gc = tick_clock.global_clock
        for p in range(N_PROCS):
            tick = gc[p]
            if tick <= 0:
                continue
            sem = sems.get(p)
            if sem is None:
                continue
            self.nc.sync.wait_ge(sem, _br.tick_to_sem(tick, p))
        self.nc.sync.drain()
        self.nc.all_engine_barrier()
        assert self.sems is not None
        popped = self.nc._tile_sem_poison_stack.pop()
        assert popped is self._sem_poison
        self.nc.clear_and_free_semaphores(list(self.sems.allocated().values()))
        self.nc.all_engine_barrier()

    _tile.TileContext._drain_and_barrier = _patched_drain_and_barrier


def _build():
    import concourse.bass as bass
    import concourse.tile as tile
    from concourse import mybir

    _apply_patches()

    F32 = mybir.dt.float32
    F16 = mybir.dt.float16
    COPY = mybir.ActivationFunctionType.Copy
    EXP = mybir.ActivationFunctionType.Exp

    nc = bass.Bass()
    x_d = nc.declare_dram_parameter("x", [BPC * N, H], F32, isOutput=False)
    qT_d = nc.declare_dram_parameter("seedT", [H, Q], F32, isOutput=False)
    id_d = nc.declare_dram_parameter("ident", [128, 128], F32, isOutput=False)
    out_d = nc.declare_dram_parameter("out", [BPC * Q, H], F32, isOutput=True)

    with tile.TileContext(nc) as tc:
        with (
            tc.tile_pool(name="const", bufs=1) as cpool,
            tc.tile_pool(name="xt", bufs=3) as xtpool,
            tc.tile_pool(name="sc", bufs=2) as scpool,
            tc.tile_pool(name="work", bufs=1) as wpool,
            tc.tile_pool(name="ps_tp", bufs=2, space="PSUM") as ps_tp,
            tc.tile_pool(name="ps_m", bufs=2, space="PSUM") as ps_m,
            tc.tile_pool(name="ps_2", bufs=1, space="PSUM") as ps_2,
        ):
            id_t = cpool.tile([128, 128], F32)
            nc.sync.dma_start(id_t[:], id_d[:])
            id16_t = cpool.tile([128, 128], F16)
            nc.vector.tensor_copy(id16_t[:], id_t[:])

            qT_f32 = cpool.tile([128, KB * 32], F32)
            for k in range(KB):
                nc.sync.dma_start(
                    qT_f32[:, 32 * k:32 * k + 32], qT_d[128 * k:128 * k + 128, :]
                )
            qT_t = cpool.tile([128, KB * 32], F16)
            nc.vector.tensor_copy(qT_t[:], qT_f32[:])

            res_t = wpool.tile([128, NCH * CW], F16)
            nc.vector.memset(res_t[:, 0:NCH * CW:CW], 1.0)
            res2_t = wpool.tile([128, OVER * CW], F16)
            nc.vector.memset(res2_t[:, 0:OVER * CW:CW], 1.0)
            strip_sb = [
                wpool.tile([32, 512], F32, name=f"strip{i}") for i in range(2)
            ]

            scratch_t = wpool.tile([128, 2048], F32)
            ge_t = wpool.tile([128, 2048], F16)
            cand_t = wpool.tile([128, 4 * WTOP], F32)
            cand2_t = wpool.tile([32, 16 * WTOP], F32)
            top_t = wpool.tile([32, 128], F32)
            mth_t = wpool.tile([32, 2], F32)
            b_mth = wpool.tile([128, 2], F32)
            negcm = wpool.tile([128, 1], F32)
            rz_t = wpool.tile([32, 1], F32)
            o2_t = wpool.tile([32, H], F32)
            wT_sb = [
                wpool.tile([128, 512], F16, name=f"wT_sb{u}") for u in range(4)
            ]

            def res_chunk(bb, c):
                """Residency slice [128, CW] for chunk c of batch bb."""
                if bb % 2 == 1 and c < OVER:
                    return res2_t[:, CW * c:CW * c + CW]
                return res_t[:, CW * c:CW * c + CW]

            def res_win(bb, w):
                """Residency slice [128, 4*CW] for window w of batch bb
                (windows never straddle the res/res2 boundary)."""
                c0 = 4 * w
                if bb % 2 == 1 and c0 < OVER:
                    return res2_t[:, CW * c0:CW * (c0 + 4)]
                return res_t[:, CW * c0:CW * (c0 + 4)]

            for b in range(BPC):
                row0 = b * N
                sc_t = scpool.tile([128, 2048], F32, name="scores")
                # ---- Phase A: stream windows (casting DMA), PE
                #      transpose fp16 chunks, fp16 mm1 into the permuted
                #      quarter layout (window w -> strip w%4, col group
                #      w//4), per-group top-24 on DVE after each group.
                for w in range(WPB):
                    t = w // 4          # column group
                    jq = w % 4          # partition strip
                    # casting DMA (SWDGE): fp32 HBM -> fp16 residency
                    nc.gpsimd.dma_start(
                        res_win(b, w).rearrange(
                            "p (c e) -> p c e", c=4
                        )[:, :, 1:1 + H],
                        x_d[row0 + 512 * w:row0 + 512 * w + 512, :].rearrange(
                            "(c p) h -> p c h", p=128
                        ),
                    )
                    pw = ps_m.tile([32, 512], F32, name="pw")
                    xt = xtpool.tile([128, KB * 512], F16)
                    for cw in range(4):
                        c = 4 * w + cw
                        src = res_chunk(b, c)[:, 1:1 + H]
                        # transpose 6 h-blocks of the fp16 chunk
                        tp = ps_tp.tile([128, KB, 128], F16, name="tp")
                        for k in range(KB):
                            nc.tensor.matmul(
                                tp[:, k, :],
                                src[:, 128 * k:128 * k + 128],
                                id16_t[:],
                                is_transpose=True, start=True, stop=True,
                                skip_group_check=True,
                            )
                        dst = xt[:, 0:KB * 512].rearrange(
                            "p (k i) -> p k i", k=KB
                        )[:, :, 128 * cw:128 * cw + 128]
                        nc.scalar.activation(dst, tp[:], COPY)
                        # mm1: 6 fp16 matmuls, M=32 at partition 0
                        for k in range(KB):
                            nc.tensor.matmul(
                                pw[:, 128 * cw:128 * cw + 128],
                                qT_t[:, 32 * k:32 * k + 32],
                                xt[:, 512 * k + 128 * cw:
                                   512 * k + 128 * cw + 128],
                                start=(k == 0), stop=(k == KB - 1),
                                skip_group_check=True,
                            )
                    # scores strip: PSUM -> SBUF staging (ACT), then a
                    # small SBUF->SBUF DMA places it on partition strip
                    # 32*jq of the scores tile (engines can't cross
                    # partitions; DMA can't read PSUM).
                    cs = slice(512 * t, 512 * t + 512)
                    if jq == 0:
                        nc.scalar.activation(sc_t[0:32, cs], pw[:], COPY)
                    else:
                        sb = strip_sb[w % 2]
                        nc.scalar.activation(sb[:], pw[:], COPY)
                        nc.sync.dma_start(
                            sc_t[32 * jq:32 * jq + 32, cs], sb[:]
                        )
                    if jq == 3:
                        # column group complete: top-24 per row on [128,512]
                        cnd = cand_t[:, WTOP * t:WTOP * t + WTOP]
                        nc.vector.max(cnd[:, 0:8], sc_t[:, cs])
                        nc.vector.match_replace(
                            scratch_t[:, cs], cnd[:, 0:8], sc_t[:, cs], NEG
                        )
                        nc.vector.max(cnd[:, 8:16], scratch_t[:, cs])
                        nc.vector.match_replace(
                            scratch_t[:, cs], cnd[:, 8:16],
                            scratch_t[:, cs], NEG
                        )
                        nc.vector.max(cnd[:, 16:24], scratch_t[:, cs])

                # ---- Phase A2: L2 exact top-128 of 384 candidates ->
                #      theta, m; w = 1[s>=theta]*exp(c(s-m))
                L1W = 4 * WTOP
                for jj in range(4):
                    nc.sync.dma_start(
                        cand2_t[:, L1W * jj:L1W * jj + L1W],
                        cand_t[32 * jj:32 * jj + 32, :],
                    )
                for r in range(16):
                    nc.vector.max(top_t[:, 8 * r:8 * r + 8], cand2_t[:])
                    if r < 15:
                        nc.vector.match_replace(
                            cand2_t[:], top_t[:, 8 * r:8 * r + 8],
                            cand2_t[:], NEG,
                        )
                nc.vector.tensor_copy(mth_t[:, 0:1], top_t[:, 0:1])
                nc.vector.tensor_copy(mth_t[:, 1:2], top_t[:, 127:128])
                for jj in range(4):
                    nc.sync.dma_start(b_mth[32 * jj:32 * jj + 32, :], mth_t[:])
                nc.vector.tensor_scalar_mul(negcm[:], b_mth[:, 0:1], -CSCALE)
                nc.scalar.activation(
                    scratch_t[:], sc_t[:], EXP,
                    bias=negcm[:], scale=CSCALE,
                )
                nc.vector.tensor_scalar(
                    ge_t[:], sc_t[:], b_mth[:, 1:2], None,
                    mybir.AluOpType.is_ge,
                )
                nc.vector.tensor_mul(sc_t[:], scratch_t[:], ge_t[:])

                # ---- Phase B: wT = transpose(w) -> fp16
                for u in range(4):
                    wtp = ps_2.tile([128, 512], F32, name="wtp")
                    for tt in range(4):
                        t = 4 * u + tt
                        nc.tensor.matmul(
                            wtp[:, 128 * tt:128 * tt + 128],
                            sc_t[:, 128 * t:128 * t + 128],
                            id_t[:],
                            is_transpose=True, start=True, stop=True,
                            skip_group_check=True,
                        )
                    nc.scalar.activation(wT_sb[u][:], wtp[:], COPY)

                # ---- Phase C: mm2 out = (w @ x) / Z, Z from ones column
                # chunk i is window w=i//4 (scores strip (i//4)%4, col
                # group i//16), within-window block i%4.
                p2a = ps_2.tile([32, 385], F32)
                p2b = ps_2.tile([32, 384], F32)
                for i in range(NCH):
                    u = i // 16
                    tt = i % 4
                    jq = (i // 4) % 4
                    lhs = wT_sb[u][:, 128 * tt + 32 * jq:128 * tt + 32 * jq + 32]
                    src = res_chunk(b, i)
                    nc.tensor.matmul(
                        p2a[:], lhs, src[:, 0:385],
                        start=(i == 0), stop=(i == NCH - 1),
                        skip_group_check=True,
                    )
                    nc.tensor.matmul(
                        p2b[:], lhs, src[:, 385:CW],
                        start=(i == 0), stop=(i == NCH - 1),
                        skip_group_check=True,
                    )
                nc.vector.reciprocal(rz_t[:], p2a[:, 0:1])
                nc.scalar.activation(
                    o2_t[:, 0:384], p2a[:, 1:385], COPY, scale=rz_t[:]
                )
                nc.scalar.activation(
                    o2_t[:, 384:768], p2b[:], COPY, scale=rz_t[:]
                )
                nc.sync.dma_start(out_d[Q * b:Q * b + Q, :], o2_t[:])

    # Split multi-wait instructions to the TRN2 1-wait-per-instruction limit
    # (the standard Bacc.compile() passes, skipped on the bass2jax run path).
    import bass_rust as _bass_rust
    _bass_rust.move_matmul_waits_to_ldweights(nc.m)
    _bass_rust.generate_event_semaphores(nc)
    return nc


def _get_nc():
    global _built
    if _built is None:
        _built = _build()
    return _built


def run(inputs, trace=False):
    from concourse.bass_utils import run_bass_kernel_spmd

    x = np.ascontiguousarray(np.asarray(inputs["input"], dtype=np.float32))
    seed = np.ascontiguousarray(np.asarray(inputs["seed"], dtype=np.float32))
    nc = _get_nc()
    seedT = np.ascontiguousarray(seed[0].T)
    ident = np.eye(128, dtype=np.float32)
    in_maps = []
    for c in range(NCORES):
        xb = np.ascontiguousarray(
            x[BPC * c:BPC * (c + 1)].reshape(BPC * N, H)
        )
        in_maps.append({"x": xb, "seedT": seedT, "ident": ident})
    res = run_bass_kernel_spmd(nc, in_maps, list(range(NCORES)), trace=trace)
    out = np.empty((B, Q, H), np.float32)
    for c in range(NCORES):
        out[BPC * c:BPC * (c + 1)] = res.results[c]["out"].reshape(BPC, Q, H)
    return out, res


def kernel(**inputs):
    out, _ = run(inputs)
    return out
